# revision 23
# baseline (speedup 1.0000x reference)
"""GQA dense-transformer block (RMSNorm + QKV + RoPE + causal GQA attention
+ o_proj + residual) on 8 trn2 NeuronCores.

Sharding: 2 (batch) x 4 (head-group tensor parallel). Core c = 4*b + g handles
batch b, q-heads 8g..8g+7, kv-heads 2g..2g+1. Each core produces a partial
o_proj output (feature-major [D, S], bf16); the host sums the 4 partials per
batch, upcasts, transposes, and adds nothing else (the RMS-normed residual is
included on g==0 cores via rmsw; g!=0 cores get rmsw=0).

Pipeline (causal): one fused loop over the 4 token windows w:
  proj(w)   PE   Q^T/K^T feature-major + V token-major (bf16 matmuls)
  stats(w)  DVE  per-token rsqrt(mean x^2 + eps) via bn_stats on bf16 x tiles
  rope(w)   DVE  raw rotate-half tables; per-token scale s folded into
                 Q^T (tensor mul with sbc) and into exp (per-key activation
                 scale) and into V (tensor_scalar on PSUM)
  oproj(w-1) PE  o_proj + residual, feature-major out (host transposes)
  attn(w)   PE/ACT/DVE  scores -> exp(scale=s_k) -> mask -> PV with a
                 1-iteration software lookahead so the PE never head-of-line
                 blocks on the exp chain
"""

import math
import numpy as np

# model dims (hardcoded per contract)
B, S, D = 2, 2048, 2048
HQ, HKV, HD = 32, 8, 64
NC = 8
NG = 4            # head groups
QH = 8            # q heads per core
KH = 2            # kv heads per core
CQ = QH * HD      # 512 q cols per core
W512 = S // 512   # 4 token windows
NT = S // 128     # 16 token tiles
NDC = D // 128    # 16 contraction chunks
PERM = [0, 4, 1, 5, 2, 6, 3, 7]  # local head order: ptile p = (h=p | h=p+4)

_cache = {}
_patched = [False]


def _legalize_bir_bytes(bir):
    """Walrus in this container accepts at most ONE embedded sem-wait per TPB
    instruction ("Too many sync wait commands"). Tile emits several when an
    instruction depends on multiple DMA queues. Split the extras into
    standalone EventSemaphore (pure-wait) instructions on the same engine
    immediately before the instruction — identical blocking semantics."""
    import json
    d = json.loads(bir if isinstance(bir, str) else bir.decode())
    n_split = 0
    stack = [d]
    while stack:
        o = stack.pop()
        if isinstance(o, dict):
            insts = o.get("instructions")
            if isinstance(insts, list) and insts and isinstance(insts[0], dict) \
               and "opcode" in insts[0]:
                new = []
                for inst in insts:
                    si = inst.get("sync_info") or {}
                    ws = si.get("on_wait") or []
                    if len(ws) > 1 and isinstance(inst.get("opcode"), str) \
                       and inst.get("opcode") not in (
                            "EventSemaphore", "UnconditionalBranch",
                            "Call", "ISA"):
                        for k, w in enumerate(ws[:-1]):
                            n_split += 1
                            new.append({
                                "debug": inst.get("debug", 0),
                                "engine": inst["engine"],
                                "ins": [], "outs": [],
                                "name": f"lw{n_split}_{inst['name']}",
                                "opcode": "EventSemaphore",
                                "sync_info": {"on_update": [], "on_wait": [w]},
                            })
                        si["on_wait"] = [ws[-1]]
                    new.append(inst)
                o["instructions"] = new
            else:
                stack.extend(o.values())
        elif isinstance(o, list):
            stack.extend(o)
    return json.dumps(d).encode()


def _install_patch():
    if _patched[0]:
        return
    from concourse import bass_utils as bu
    from concourse import bass2jax as b2j
    orig = bu.compile_bir_kernel

    def patched(bir, *a, **k):
        return orig(_legalize_bir_bytes(bir), *a, **k)

    bu.compile_bir_kernel = patched
    b2j.compile_bir_kernel = patched
    _patched[0] = True


def _build(causal: bool):
    import concourse.bass as bass
    import concourse.mybir as mybir
    from concourse.tile import TileContext

    fp32 = mybir.dt.float32
    bf16 = mybir.dt.bfloat16
    AF = mybir.ActivationFunctionType
    DIV = mybir.AluOpType.divide

    nc = bass.Bass("TRN2")
    xT = nc.dram_tensor("xT", [D, S], bf16, kind="ExternalInput")
    xb_d = nc.dram_tensor("xb", [S, D], bf16, kind="ExternalInput")
    wq_d = nc.dram_tensor("wq", [128, NDC * CQ], bf16, kind="ExternalInput")
    wk_d = nc.dram_tensor("wk", [128, NDC * 128], bf16, kind="ExternalInput")
    wv_d = nc.dram_tensor("wv", [128, NDC * 128], bf16, kind="ExternalInput")
    wo_d = nc.dram_tensor("wo", [128, 4 * D], bf16, kind="ExternalInput")
    cos_d = nc.dram_tensor("cosT", [128, S], bf16, kind="ExternalInput")
    sin_d = nc.dram_tensor("sinT", [128, S], bf16, kind="ExternalInput")
    maskb_d = nc.dram_tensor("maskb", [128, 896], bf16, kind="ExternalInput")
    rmsw_d = nc.dram_tensor("rmsw", [128, NDC], fp32, kind="ExternalInput")
    out = nc.dram_tensor("out", [D, S], bf16, kind="ExternalOutput")

    with TileContext(nc) as tc:
        with (
            nc.allow_low_precision(reason="bf16 pipeline within 2e-2 tolerance"),
            tc.tile_pool(name="res", bufs=1) as res,
            tc.tile_pool(name="dram", bufs=1, space="DRAM") as dpool,
            tc.tile_pool(name="xtw_p", bufs=2) as xtw_p,
            tc.tile_pool(name="obw_p", bufs=2) as obw_p,
            tc.tile_pool(name="stat", bufs=3) as sp,
            tc.tile_pool(name="srow_p", bufs=2) as srow_p,
            tc.tile_pool(name="rtmp", bufs=3) as rtmp,
            tc.tile_pool(name="aex", bufs=6) as aex,
            tc.tile_pool(name="asm", bufs=2) as asm,
            tc.tile_pool(name="oep", bufs=4) as oep,
        ):
            # ---- resident tiles ----
            QT = [res.tile([128, S], bf16, tag=f"qt{p}", name=f"qt{p}") for p in range(4)]
            KT = res.tile([128, S], bf16, tag="kt", name="kt")
            AT = [res.tile([128, S], bf16, tag=f"at{p}", name=f"at{p}") for p in range(4)]
            v_all = res.tile([128, NT * 130], bf16, tag="vall", name="vall")
            cosb = res.tile([128, S], bf16, tag="cosb", name="cosb")
            sinb = res.tile([128, S], bf16, tag="sinb", name="sinb")
            sbc_b = res.tile([128, S], bf16, tag="sbc", name="sbc_b")
            maskb = res.tile([128, 896], bf16, tag="maskb", name="maskb")
            rmswT = res.tile([128, NDC], fp32, tag="rmsw", name="rmswT")
            s_all = res.tile([128, NT], fp32, tag="sall", name="s_all")
            wq_r = res.tile([128, NDC * CQ], bf16, tag="wqr", name="wq_r")
            wk_r = res.tile([128, NDC * 128], bf16, tag="wkr", name="wk_r")
            wv_r = res.tile([128, NDC * 128], bf16, tag="wvr", name="wv_r")
            wo_r = res.tile([128, 4 * D], bf16, tag="wor", name="wo_r")
            ones1b = res.tile([1, 128], bf16, tag="ones1b", name="ones1b")
            epst = res.tile([128, 1], fp32, tag="epst", name="epst")
            ones_c = res.tile([128, 1], fp32, tag="onesc", name="ones_c")
            s_dram = dpool.tile([S, 1], fp32, tag="sdram", name="s_dram")

            nc.vector.memset(ones1b[:, :], 1.0)
            nc.vector.memset(epst[:, :], float(np.finfo(np.float32).eps))
            nc.vector.memset(ones_c[:, :], 1.0)
            nc.vector.memset(v_all[:, :], 1.0)
            nc.gpsimd.dma_start(out=cosb[:, :], in_=cos_d[:, :])
            nc.gpsimd.dma_start(out=sinb[:, :], in_=sin_d[:, :])
            nc.gpsimd.dma_start(out=maskb[:, :], in_=maskb_d[:, :])
            nc.gpsimd.dma_start(out=rmswT[:, :], in_=rmsw_d[:, :])
            nc.gpsimd.dma_start(out=wq_r[:, :], in_=wq_d[:, :])
            nc.gpsimd.dma_start(out=wk_r[:, :], in_=wk_d[:, :])
            nc.gpsimd.dma_start(out=wv_r[:, :], in_=wv_d[:, :])
            nc.gpsimd.dma_start(out=wo_r[:, :], in_=wo_d[:, :])

            has_pool_mul = hasattr(nc.gpsimd, "tensor_mul")
            has_pool_copy = hasattr(nc.gpsimd, "tensor_copy")

            xtw = {}

            def load_xtw(w):
                t = xtw_p.tile([128, NDC * 512], bf16, tag="xtw", name="xtw")
                wsl = slice(512 * w, 512 * (w + 1))
                nc.gpsimd.dma_start(
                    out=t[:, :].rearrange("p (c t) -> p c t", c=NDC),
                    in_=xT[:, wsl].rearrange("(c p) t -> p c t", p=128))
                xtw[w] = t

            def proj_w(w, pools):
                pq, pk, pv_ = pools
                qs = [pq.tile([128, 512], fp32, tag="psq", name="psq") for _ in range(4)]
                ks = pk.tile([128, 512], fp32, tag="psk", name="psk")
                vs = pv_.tile([128, 512], fp32, tag="psv", name="psv")
                xt = xtw[w]
                for dc in range(NDC):
                    xsl = slice(512 * dc, 512 * (dc + 1))
                    st_, sp_ = (dc == 0), (dc == NDC - 1)
                    for ct in range(4):
                        nc.tensor.matmul(
                            qs[ct][:, :],
                            wq_r[:, dc * CQ + ct * 128 : dc * CQ + (ct + 1) * 128],
                            xt[:, xsl], start=st_, stop=sp_)
                    nc.tensor.matmul(ks[:, :], wk_r[:, dc * 128 : (dc + 1) * 128],
                                     xt[:, xsl], start=st_, stop=sp_)
                    # four V slice-groups share one PSUM bank; start=True resets
                    # the whole bank so only the first matmul may carry it
                    for vt in range(4):
                        nc.tensor.matmul(
                            vs[:, 128 * vt : 128 * (vt + 1)],
                            xt[:, 512 * dc + 128 * vt : 512 * dc + 128 * (vt + 1)],
                            wv_r[:, dc * 128 : (dc + 1) * 128],
                            start=(st_ and vt == 0), stop=sp_,
                            skip_group_check=True)
                return qs, ks, vs

            def stats_w(w, pbb):
                # per-token 1/sqrt(mean(x^2)+eps) for tokens of window w
                sq_w = sp.tile([128, 4], fp32, tag="sqw", name="sq_w")
                for vt in range(4):
                    tt = 4 * w + vt
                    x_t = sp.tile([128, D], bf16, tag="xs", name="xs")
                    nc.gpsimd.dma_start(out=x_t[:, :],
                                        in_=xb_d[tt * 128 : (tt + 1) * 128, :])
                    st = sp.tile([128, 4, 6], fp32, tag="st", name="st")
                    for c in range(4):
                        nc.vector.bn_stats(out=st[:, c, :], in_=x_t[:, 512 * c : 512 * (c + 1)])
                    mv = sp.tile([128, 2], fp32, tag="mv", name="mv")
                    nc.vector.bn_aggr(out=mv[:, :], in_=st[:, :, :])
                    msq = sp.tile([128, 1], fp32, tag="msq", name="msq")
                    nc.vector.tensor_mul(msq[:, :], mv[:, 0:1], mv[:, 0:1])
                    nc.vector.tensor_add(msq[:, :], msq[:, :], mv[:, 1:2])
                    nc.scalar.activation(out=sq_w[:, vt : vt + 1], in_=msq[:, :],
                                         func=AF.Sqrt, bias=epst[:, 0:1], scale=1.0)
                nc.vector.reciprocal(out=s_all[:, 4 * w : 4 * (w + 1)], in_=sq_w[:, :])
                # bounce s (token-within-tile-major) -> flat row, broadcast to sbc
                wsl = slice(512 * w, 512 * (w + 1))
                nc.gpsimd.dma_start(
                    out=s_dram[wsl, :].rearrange("(t p) one -> p (t one)", p=128),
                    in_=s_all[:, 4 * w : 4 * (w + 1)])
                s_row = srow_p.tile([1, 512], fp32, tag="srow", name="s_row")
                nc.gpsimd.dma_start(out=s_row[0:1, :],
                                    in_=s_dram[wsl, :].rearrange("s one -> one s"))
                s_rowb = srow_p.tile([1, 512], bf16, tag="srowb", name="s_rowb")
                nc.vector.tensor_copy(s_rowb[:, :], s_row[:, :])
                psb = pbb.tile([128, 512], fp32, tag="psb", name="psb")
                nc.tensor.matmul(psb[:, :], ones1b[0:1, :], s_rowb[0:1, :],
                                 start=True, stop=True)
                nc.scalar.copy(out=sbc_b[:, wsl], in_=psb[:, :])

            def rope_w(w, qs, ks, vs):
                wsl = slice(512 * w, 512 * (w + 1))
                # KT first so attention can start as early as possible
                for ct in (4, 0, 1, 2, 3):
                    src = ks if ct == 4 else qs[ct]
                    dst = KT if ct == 4 else QT[ct]
                    tmp = rtmp.tile([128, 512], bf16, tag="rt", name="rt")
                    for a, bidx in ((0, 1), (1, 0), (2, 3), (3, 2)):
                        nc.vector.tensor_mul(tmp[32 * a : 32 * (a + 1), :],
                                             src[32 * bidx : 32 * (bidx + 1), :],
                                             sinb[32 * a : 32 * (a + 1), wsl])
                    nc.vector.tensor_mul(dst[:, wsl], src[:, :], cosb[:, wsl])
                    nc.vector.tensor_add(dst[:, wsl], dst[:, wsl], tmp[:, :])
                    # fold per-token scale into both Q^T (s_q) and K^T (s_k)
                    nc.vector.tensor_mul(dst[:, wsl], dst[:, wsl], sbc_b[:, wsl])
                # scale V by per-token s and write into v_all
                for vt in range(4):
                    tt = 4 * w + vt
                    for h in range(2):
                        nc.vector.tensor_scalar_mul(
                            v_all[:, 130 * tt + 65 * h : 130 * tt + 65 * h + 64],
                            vs[:, 128 * vt + 64 * h : 128 * vt + 64 * (h + 1)],
                            s_all[:, tt : tt + 1])

            def attn_w(w, pools):
                psc, ppv, pbc = pools
                kt_max = 4 * (w + 1) if causal else NT
                wsl = slice(512 * w, 512 * (w + 1))
                for p in range(4):
                    pvs = [ppv.tile([65, 512], fp32, tag="pv", name="pv") for _ in range(2)]
                    exq = []

                    def pv_pair(kt):
                        ex2 = exq[kt]
                        for h in range(2):
                            nc.tensor.matmul(
                                pvs[h][:, :],
                                v_all[:, 130 * kt + 65 * h : 130 * kt + 65 * (h + 1)],
                                ex2[:, 512 * h : 512 * (h + 1)],
                                start=(kt == 0), stop=(kt == kt_max - 1))

                    for kt in range(kt_max):
                        dd = 128 * kt - 512 * w
                        sc2 = psc.tile([128, 1024], fp32, tag="sc2", name="sc2")
                        for h in range(2):
                            nc.tensor.matmul(
                                sc2[:, 512 * h : 512 * (h + 1)],
                                KT[64 * h : 64 * (h + 1), kt * 128 : (kt + 1) * 128],
                                QT[p][64 * h : 64 * (h + 1), wsl],
                                start=True, stop=True)
                        ex2 = aex.tile([128, 1024], bf16, tag="ex", name="ex")
                        nc.scalar.activation(out=ex2[:, :], in_=sc2[:, :], func=AF.Exp)
                        if causal and 0 <= dd <= 384:
                            off = 384 - dd
                            ex2v = ex2[:, :].rearrange("p (a b) -> p a b", a=2)
                            mrep = maskb[:, off : off + 512].rearrange(
                                "p (a f) -> p a f", a=1).to_broadcast((128, 2, 512))
                            nc.vector.tensor_mul(ex2v, ex2v, mrep)
                        exq.append(ex2)
                        if kt >= 1:
                            pv_pair(kt - 1)
                    pv_pair(kt_max - 1)

                    for h in range(2):
                        inv = asm.tile([1, 512], bf16, tag="inv", name="inv")
                        nc.vector.reciprocal(out=inv[:, :], in_=pvs[h][64:65, :])
                        bcp = pbc.tile([64, 512], fp32, tag="bcp", name="bcp")
                        nc.tensor.matmul(bcp[:, :], ones1b[0:1, 0:64], inv[0:1, :],
                                         start=True, stop=True)
                        bc = asm.tile([64, 512], bf16, tag="bc", name="bc")
                        nc.scalar.copy(out=bc[:, :], in_=bcp[:, :])
                        nc.vector.tensor_mul(AT[p][64 * h : 64 * (h + 1), wsl],
                                             pvs[h][0:64, :], bc[:, :])

            def oproj_w(w, po, xt=None, dc_rng=None, obw=None):
                wsl = slice(512 * w, 512 * (w + 1))
                if obw is None:
                    obw = obw_p.tile([128, NDC * 512], bf16, tag="obw", name="obw")
                if xt is None:
                    xt = xtw[w]
                if dc_rng is None:
                    dc_rng = range(NDC)
                for dc in dc_rng:
                    pso = po.tile([128, 512], fp32, tag="pso", name="pso")
                    for c in range(4):
                        nc.tensor.matmul(
                            pso[:, :],
                            wo_r[:, c * D + dc * 128 : c * D + (dc + 1) * 128],
                            AT[c][:, wsl], start=(c == 0), stop=(c == 3))
                    xsl = slice(512 * dc, 512 * (dc + 1))
                    t1 = oep.tile([128, 512], bf16, tag="t1", name="t1")
                    if has_pool_mul:
                        nc.gpsimd.tensor_mul(t1[:, :], xt[:, xsl], sbc_b[:, wsl])
                    else:
                        nc.vector.tensor_mul(t1[:, :], xt[:, xsl], sbc_b[:, wsl])
                    t2 = oep.tile([128, 512], bf16, tag="t2", name="t2")
                    nc.vector.tensor_scalar_mul(t2[:, :], t1[:, :], rmswT[:, dc : dc + 1])
                    nc.vector.tensor_add(obw[:, xsl], t2[:, :], pso[:, :])
                if dc_rng.stop == NDC:
                    nc.gpsimd.dma_start(
                        out=out[:, wsl].rearrange("(c p) t -> p c t", p=128),
                        in_=obw[:, :].rearrange("p (c t) -> p c t", c=NDC))
                return obw

            # ---- schedule ----
            load_xtw(0)
            if causal:
                for w in range(W512):
                    obw_prev = None
                    if w > 0:
                        with tc.tile_pool(name="po1", bufs=4, space="PSUM") as po1:
                            obw_prev = oproj_w(w - 1, po1, dc_rng=range(0, 8))
                    with (
                        tc.tile_pool(name="pq", bufs=4, space="PSUM") as pq,
                        tc.tile_pool(name="pkb", bufs=1, space="PSUM") as pkb,
                        tc.tile_pool(name="pv", bufs=1, space="PSUM") as pv_,
                        tc.tile_pool(name="po2", bufs=1, space="PSUM") as po2,
                    ):
                        qs, ks, vs = proj_w(w, (pq, pkb, pv_))
                        if w + 1 < W512:
                            load_xtw(w + 1)
                        stats_w(w, pkb)
                        rope_w(w, qs, ks, vs)
                        if w > 0:
                            # second oproj half fills the PE while RoPE runs
                            oproj_w(w - 1, po2, dc_rng=range(8, NDC), obw=obw_prev)
                    with (
                        tc.tile_pool(name="psc", bufs=2, space="PSUM") as psc,
                        tc.tile_pool(name="ppv", bufs=3, space="PSUM") as ppv,
                        tc.tile_pool(name="pbc", bufs=1, space="PSUM") as pbc,
                    ):
                        attn_w(w, (psc, ppv, pbc))
                with tc.tile_pool(name="po", bufs=4, space="PSUM") as po:
                    oproj_w(W512 - 1, po)
            else:
                for w in range(W512):
                    with (
                        tc.tile_pool(name="pq", bufs=4, space="PSUM") as pq,
                        tc.tile_pool(name="pkb", bufs=1, space="PSUM") as pkb,
                        tc.tile_pool(name="pv", bufs=1, space="PSUM") as pv_,
                    ):
                        qs, ks, vs = proj_w(w, (pq, pkb, pv_))
                        if w + 1 < W512:
                            load_xtw(w + 1)
                        stats_w(w, pkb)
                        rope_w(w, qs, ks, vs)
                # all windows' K/V needed before any attention window
                for w in range(W512):
                    with (
                        tc.tile_pool(name="psc", bufs=2, space="PSUM") as psc,
                        tc.tile_pool(name="ppv", bufs=3, space="PSUM") as ppv,
                        tc.tile_pool(name="pbc", bufs=1, space="PSUM") as pbc,
                    ):
                        attn_w(w, (psc, ppv, pbc))
                    # reload x^T for the residual (proj-time tiles are evicted)
                    ox = obw_p.tile([128, NDC * 512], bf16, tag="oxw", name="oxw")
                    wsl = slice(512 * w, 512 * (w + 1))
                    nc.gpsimd.dma_start(
                        out=ox[:, :].rearrange("p (c t) -> p c t", c=NDC),
                        in_=xT[:, wsl].rearrange("(c p) t -> p c t", p=128))
                    with tc.tile_pool(name="po", bufs=4, space="PSUM") as po:
                        oproj_w(w, po, xt=ox)
    return nc


def _host_prep(x, rms_w, Wq, Wk, Wv, Wo):
    import ml_dtypes
    f32 = np.float32
    bf16 = ml_dtypes.bfloat16
    x = np.asarray(x, f32)
    rms_w = np.asarray(rms_w, f32)
    wq_full = (np.asarray(Wq, f32) * rms_w[:, None] / math.sqrt(HD)).astype(f32)
    wk_full = (np.asarray(Wk, f32) * rms_w[:, None]).astype(f32)
    wv_full = (np.asarray(Wv, f32) * rms_w[:, None]).astype(f32)
    Wo = np.asarray(Wo, f32)

    inv_f = (1.0 / (10000.0 ** (np.arange(0, HD, 2, dtype=f32) / HD))).astype(f32)
    freqs = np.arange(S, dtype=f32)[:, None] * inv_f[None, :]   # [S, 32]
    cos = np.cos(freqs).astype(f32).T                           # [32, S]
    sin = np.sin(freqs).astype(f32).T
    cosT = np.tile(np.concatenate([cos, cos], 0), (2, 1))       # [128, S]
    sinT = np.tile(np.concatenate([-sin, sin], 0), (2, 1))

    kk = np.arange(128)[:, None]
    jj = np.arange(896)[None, :]
    maskb = (jj >= kk + 384).astype(f32)

    per_core = []
    for c in range(NC):
        b, g = c // 4, c % 4
        heads = [8 * g + h for h in PERM]
        wq_g = np.ascontiguousarray(
            np.concatenate([wq_full[:, 64 * h : 64 * (h + 1)] for h in heads], axis=1))
        wo_g = np.ascontiguousarray(
            np.concatenate([Wo[64 * h : 64 * (h + 1), :] for h in heads], axis=0))
        wk_g = np.ascontiguousarray(wk_full[:, 128 * g : 128 * (g + 1)])
        wv_g = np.ascontiguousarray(wv_full[:, 128 * g : 128 * (g + 1)])
        # chunk-major resident layouts: [128, chunk-index * cols]
        wq_r = np.ascontiguousarray(
            wq_g.reshape(NDC, 128, CQ).transpose(1, 0, 2).reshape(128, NDC * CQ))
        wk_r = np.ascontiguousarray(
            wk_g.reshape(NDC, 128, 128).transpose(1, 0, 2).reshape(128, NDC * 128))
        wv_r = np.ascontiguousarray(
            wv_g.reshape(NDC, 128, 128).transpose(1, 0, 2).reshape(128, NDC * 128))
        wo_r = np.ascontiguousarray(
            wo_g.reshape(4, 128, D).transpose(1, 0, 2).reshape(128, 4 * D))
        rmsw_g = rms_w if g == 0 else np.zeros((D,), f32)
        rmswT = np.ascontiguousarray(
            rmsw_g.reshape(NDC, 128).T.astype(f32))        # [128, NDC]
        xb = x[b].astype(bf16)
        per_core.append({
            "xT": np.ascontiguousarray(xb.T),
            "xb": np.ascontiguousarray(xb),
            "wq": wq_r.astype(bf16), "wk": wk_r.astype(bf16),
            "wv": wv_r.astype(bf16), "wo": wo_r.astype(bf16),
            "cosT": np.ascontiguousarray(cosT.astype(bf16)),
            "sinT": np.ascontiguousarray(sinT.astype(bf16)),
            "maskb": maskb.astype(bf16), "rmsw": rmswT,
        })
    return per_core


def kernel(x, rms_w, Wq, Wk, Wv, Wo, apply_causal_mask, _trace=False):
    from concourse import bass_utils
    _install_patch()
    causal = bool(int(np.asarray(apply_causal_mask)))
    if causal not in _cache:
        _cache[causal] = _build(causal)
    nc = _cache[causal]
    in_maps = _host_prep(x, rms_w, Wq, Wk, Wv, Wo)
    r = bass_utils.run_bass_kernel_spmd(nc, in_maps, core_ids=list(range(NC)),
                                        trace=_trace)
    outs = [np.asarray(r.results[c]["out"], dtype=np.float32) for c in range(NC)]
    full = np.stack([(outs[4 * b] + outs[4 * b + 1] + outs[4 * b + 2] + outs[4 * b + 3]).T
                     for b in range(B)]).astype(np.float32)
    if _trace:
        kernel.last_exec_time_ns = r.exec_time_ns
        kernel.last_result = r
    return full


# revision 24
# speedup vs baseline: 1.1365x; 1.1365x over previous
"""GQA dense-transformer block (RMSNorm + QKV + RoPE + causal GQA attention
+ o_proj + residual) on 8 trn2 NeuronCores.

Sharding: 2 (batch) x 4 (head-group tensor parallel). Core c = 4*b + g handles
batch b, q-heads 8g..8g+7, kv-heads 2g..2g+1. Each core produces a partial
o_proj output (feature-major [D, S], bf16); the host sums the 4 partials per
batch, upcasts, transposes, and adds nothing else (the RMS-normed residual is
included on g==0 cores via rmsw; g!=0 cores get rmsw=0).

Pipeline (causal): one fused loop over the 4 token windows w:
  proj(w)   PE   Q^T/K^T feature-major + V token-major (bf16 matmuls)
  stats(w)  DVE  per-token rsqrt(mean x^2 + eps) via bn_stats on bf16 x tiles
  rope(w)   DVE  raw rotate-half tables; per-token scale s folded into
                 Q^T (tensor mul with sbc) and into exp (per-key activation
                 scale) and into V (tensor_scalar on PSUM)
  oproj(w-1) PE  o_proj + residual, feature-major out (host transposes)
  attn(w)   PE/ACT/DVE  scores -> exp(scale=s_k) -> mask -> PV with a
                 1-iteration software lookahead so the PE never head-of-line
                 blocks on the exp chain
"""

import math
import numpy as np

# model dims (hardcoded per contract)
B, S, D = 2, 2048, 2048
HQ, HKV, HD = 32, 8, 64
NC = 8
NG = 4            # head groups
QH = 8            # q heads per core
KH = 2            # kv heads per core
CQ = QH * HD      # 512 q cols per core
W512 = S // 512   # 4 token windows
NT = S // 128     # 16 token tiles
NDC = D // 128    # 16 contraction chunks
PERM = [0, 4, 1, 5, 2, 6, 3, 7]  # local head order: ptile p = (h=p | h=p+4)

_cache = {}
_patched = [False]


def _legalize_bir_bytes(bir):
    """Walrus in this container accepts at most ONE embedded sem-wait per TPB
    instruction ("Too many sync wait commands"). Tile emits several when an
    instruction depends on multiple DMA queues. Split the extras into
    standalone EventSemaphore (pure-wait) instructions on the same engine
    immediately before the instruction — identical blocking semantics."""
    import json
    d = json.loads(bir if isinstance(bir, str) else bir.decode())
    n_split = 0
    stack = [d]
    while stack:
        o = stack.pop()
        if isinstance(o, dict):
            insts = o.get("instructions")
            if isinstance(insts, list) and insts and isinstance(insts[0], dict) \
               and "opcode" in insts[0]:
                new = []
                for inst in insts:
                    si = inst.get("sync_info") or {}
                    ws = si.get("on_wait") or []
                    if len(ws) > 1 and isinstance(inst.get("opcode"), str) \
                       and inst.get("opcode") not in (
                            "EventSemaphore", "UnconditionalBranch",
                            "Call", "ISA"):
                        for k, w in enumerate(ws[:-1]):
                            n_split += 1
                            new.append({
                                "debug": inst.get("debug", 0),
                                "engine": inst["engine"],
                                "ins": [], "outs": [],
                                "name": f"lw{n_split}_{inst['name']}",
                                "opcode": "EventSemaphore",
                                "sync_info": {"on_update": [], "on_wait": [w]},
                            })
                        si["on_wait"] = [ws[-1]]
                    new.append(inst)
                o["instructions"] = new
            else:
                stack.extend(o.values())
        elif isinstance(o, list):
            stack.extend(o)
    return json.dumps(d).encode()


def _install_patch():
    if _patched[0]:
        return
    from concourse import bass_utils as bu
    from concourse import bass2jax as b2j
    orig = bu.compile_bir_kernel

    def patched(bir, *a, **k):
        return orig(_legalize_bir_bytes(bir), *a, **k)

    bu.compile_bir_kernel = patched
    b2j.compile_bir_kernel = patched
    _patched[0] = True


def _build(causal: bool):
    import concourse.bass as bass
    import concourse.mybir as mybir
    from concourse.tile import TileContext

    fp32 = mybir.dt.float32
    bf16 = mybir.dt.bfloat16
    AF = mybir.ActivationFunctionType
    DIV = mybir.AluOpType.divide

    nc = bass.Bass("TRN2")
    xT = nc.dram_tensor("xT", [D, S], bf16, kind="ExternalInput")
    xb_d = nc.dram_tensor("xb", [S, D], bf16, kind="ExternalInput")
    wq_d = nc.dram_tensor("wq", [128, NDC * CQ], bf16, kind="ExternalInput")
    wk_d = nc.dram_tensor("wk", [128, NDC * 128], bf16, kind="ExternalInput")
    wv_d = nc.dram_tensor("wv", [128, NDC * 128], bf16, kind="ExternalInput")
    wo_d = nc.dram_tensor("wo", [128, 4 * D], bf16, kind="ExternalInput")
    cos_d = nc.dram_tensor("cosT", [128, S], bf16, kind="ExternalInput")
    sin_d = nc.dram_tensor("sinT", [128, S], bf16, kind="ExternalInput")
    maskb_d = nc.dram_tensor("maskb", [128, 896], bf16, kind="ExternalInput")
    rmsw_d = nc.dram_tensor("rmsw", [128, NDC], fp32, kind="ExternalInput")
    out = nc.dram_tensor("out", [D, S], bf16, kind="ExternalOutput")

    with TileContext(nc) as tc:
        with (
            nc.allow_low_precision(reason="bf16 pipeline within 2e-2 tolerance"),
            tc.tile_pool(name="res", bufs=1) as res,
            tc.tile_pool(name="dram", bufs=1, space="DRAM") as dpool,
            tc.tile_pool(name="xtw_p", bufs=2) as xtw_p,
            tc.tile_pool(name="obw_p", bufs=2) as obw_p,
            tc.tile_pool(name="stat", bufs=3) as sp,
            tc.tile_pool(name="srow_p", bufs=2) as srow_p,
            tc.tile_pool(name="rtmp", bufs=3) as rtmp,
            tc.tile_pool(name="aex", bufs=6) as aex,
            tc.tile_pool(name="asm", bufs=2) as asm,
            tc.tile_pool(name="oep", bufs=4) as oep,
        ):
            # ---- resident tiles ----
            QT = [res.tile([128, S], bf16, tag=f"qt{p}", name=f"qt{p}") for p in range(4)]
            KT = res.tile([128, S], bf16, tag="kt", name="kt")
            AT = [res.tile([128, S], bf16, tag=f"at{p}", name=f"at{p}") for p in range(4)]
            v_all = res.tile([128, NT * 130], bf16, tag="vall", name="vall")
            cosb = res.tile([128, S], bf16, tag="cosb", name="cosb")
            sinb = res.tile([128, S], bf16, tag="sinb", name="sinb")
            sbc_b = res.tile([128, S], bf16, tag="sbc", name="sbc_b")
            maskb = res.tile([128, 896], bf16, tag="maskb", name="maskb")
            rmswT = res.tile([128, NDC], fp32, tag="rmsw", name="rmswT")
            s_all = res.tile([128, NT], fp32, tag="sall", name="s_all")
            wq_r = res.tile([128, NDC * CQ], bf16, tag="wqr", name="wq_r")
            wk_r = res.tile([128, NDC * 128], bf16, tag="wkr", name="wk_r")
            wv_r = res.tile([128, NDC * 128], bf16, tag="wvr", name="wv_r")
            wo_r = res.tile([128, 4 * D], bf16, tag="wor", name="wo_r")
            ones1b = res.tile([1, 128], bf16, tag="ones1b", name="ones1b")
            epst = res.tile([128, 1], fp32, tag="epst", name="epst")
            ones_c = res.tile([128, 1], fp32, tag="onesc", name="ones_c")
            s_dram = dpool.tile([S, 1], fp32, tag="sdram", name="s_dram")

            nc.vector.memset(ones1b[:, :], 1.0)
            nc.vector.memset(epst[:, :], float(np.finfo(np.float32).eps))
            nc.vector.memset(ones_c[:, :], 1.0)
            nc.vector.memset(v_all[:, :], 1.0)
            nc.gpsimd.dma_start(out=cosb[:, :], in_=cos_d[:, :])
            nc.gpsimd.dma_start(out=sinb[:, :], in_=sin_d[:, :])
            nc.gpsimd.dma_start(out=maskb[:, :], in_=maskb_d[:, :])
            nc.gpsimd.dma_start(out=rmswT[:, :], in_=rmsw_d[:, :])
            nc.gpsimd.dma_start(out=wq_r[:, :], in_=wq_d[:, :])
            nc.gpsimd.dma_start(out=wk_r[:, :], in_=wk_d[:, :])
            nc.gpsimd.dma_start(out=wv_r[:, :], in_=wv_d[:, :])
            nc.gpsimd.dma_start(out=wo_r[:, :], in_=wo_d[:, :])

            has_pool_mul = hasattr(nc.gpsimd, "tensor_mul")
            has_pool_copy = hasattr(nc.gpsimd, "tensor_copy")

            xtw = {}

            def load_xtw(w):
                t = xtw_p.tile([128, NDC * 512], bf16, tag="xtw", name="xtw")
                wsl = slice(512 * w, 512 * (w + 1))
                nc.gpsimd.dma_start(
                    out=t[:, :].rearrange("p (c t) -> p c t", c=NDC),
                    in_=xT[:, wsl].rearrange("(c p) t -> p c t", p=128))
                xtw[w] = t

            def proj_w(w, pools):
                pq, pk, pv_ = pools
                qs = [pq.tile([128, 512], fp32, tag="psq", name="psq") for _ in range(4)]
                ks = pk.tile([128, 512], fp32, tag="psk", name="psk")
                vs = pv_.tile([128, 512], fp32, tag="psv", name="psv")
                xt = xtw[w]
                for dc in range(NDC):
                    xsl = slice(512 * dc, 512 * (dc + 1))
                    st_, sp_ = (dc == 0), (dc == NDC - 1)
                    for ct in range(4):
                        nc.tensor.matmul(
                            qs[ct][:, :],
                            wq_r[:, dc * CQ + ct * 128 : dc * CQ + (ct + 1) * 128],
                            xt[:, xsl], start=st_, stop=sp_)
                    nc.tensor.matmul(ks[:, :], wk_r[:, dc * 128 : (dc + 1) * 128],
                                     xt[:, xsl], start=st_, stop=sp_)
                    # four V slice-groups share one PSUM bank; start=True resets
                    # the whole bank so only the first matmul may carry it
                    for vt in range(4):
                        nc.tensor.matmul(
                            vs[:, 128 * vt : 128 * (vt + 1)],
                            xt[:, 512 * dc + 128 * vt : 512 * dc + 128 * (vt + 1)],
                            wv_r[:, dc * 128 : (dc + 1) * 128],
                            start=(st_ and vt == 0), stop=sp_,
                            skip_group_check=True)
                return qs, ks, vs

            def stats_w(w, pbb):
                # per-token 1/sqrt(mean(x^2)+eps) for tokens of window w
                sq_w = sp.tile([128, 4], fp32, tag="sqw", name="sq_w")
                for vt in range(4):
                    tt = 4 * w + vt
                    x_t = sp.tile([128, D], bf16, tag="xs", name="xs")
                    nc.gpsimd.dma_start(out=x_t[:, :],
                                        in_=xb_d[tt * 128 : (tt + 1) * 128, :])
                    st = sp.tile([128, 4, 6], fp32, tag="st", name="st")
                    for c in range(4):
                        nc.vector.bn_stats(out=st[:, c, :], in_=x_t[:, 512 * c : 512 * (c + 1)])
                    mv = sp.tile([128, 2], fp32, tag="mv", name="mv")
                    nc.vector.bn_aggr(out=mv[:, :], in_=st[:, :, :])
                    msq = sp.tile([128, 1], fp32, tag="msq", name="msq")
                    nc.vector.tensor_mul(msq[:, :], mv[:, 0:1], mv[:, 0:1])
                    nc.vector.tensor_add(msq[:, :], msq[:, :], mv[:, 1:2])
                    nc.scalar.activation(out=sq_w[:, vt : vt + 1], in_=msq[:, :],
                                         func=AF.Sqrt, bias=epst[:, 0:1], scale=1.0)
                nc.vector.reciprocal(out=s_all[:, 4 * w : 4 * (w + 1)], in_=sq_w[:, :])
                # bounce s (token-within-tile-major) -> flat row, broadcast to sbc
                wsl = slice(512 * w, 512 * (w + 1))
                nc.gpsimd.dma_start(
                    out=s_dram[wsl, :].rearrange("(t p) one -> p (t one)", p=128),
                    in_=s_all[:, 4 * w : 4 * (w + 1)])
                s_row = srow_p.tile([1, 512], fp32, tag="srow", name="s_row")
                nc.gpsimd.dma_start(out=s_row[0:1, :],
                                    in_=s_dram[wsl, :].rearrange("s one -> one s"))
                s_rowb = srow_p.tile([1, 512], bf16, tag="srowb", name="s_rowb")
                nc.vector.tensor_copy(s_rowb[:, :], s_row[:, :])
                psb = pbb.tile([128, 512], fp32, tag="psb", name="psb")
                nc.tensor.matmul(psb[:, :], ones1b[0:1, :], s_rowb[0:1, :],
                                 start=True, stop=True)
                nc.scalar.copy(out=sbc_b[:, wsl], in_=psb[:, :])

            def rope_w(w, qs, ks, vs):
                wsl = slice(512 * w, 512 * (w + 1))
                # KT first so attention can start as early as possible
                for ct in (4, 0, 1, 2, 3):
                    src = ks if ct == 4 else qs[ct]
                    dst = KT if ct == 4 else QT[ct]
                    tmp = rtmp.tile([128, 512], bf16, tag="rt", name="rt")
                    for a, bidx in ((0, 1), (1, 0), (2, 3), (3, 2)):
                        nc.vector.tensor_mul(tmp[32 * a : 32 * (a + 1), :],
                                             src[32 * bidx : 32 * (bidx + 1), :],
                                             sinb[32 * a : 32 * (a + 1), wsl])
                    nc.vector.tensor_mul(dst[:, wsl], src[:, :], cosb[:, wsl])
                    nc.vector.tensor_add(dst[:, wsl], dst[:, wsl], tmp[:, :])
                    # fold per-token scale into both Q^T (s_q) and K^T (s_k)
                    nc.vector.tensor_mul(dst[:, wsl], dst[:, wsl], sbc_b[:, wsl])
                # scale V by per-token s and write into v_all
                for vt in range(4):
                    tt = 4 * w + vt
                    for h in range(2):
                        nc.vector.tensor_scalar_mul(
                            v_all[:, 130 * tt + 65 * h : 130 * tt + 65 * h + 64],
                            vs[:, 128 * vt + 64 * h : 128 * vt + 64 * (h + 1)],
                            s_all[:, tt : tt + 1])

            def attn_w(w, pools):
                psc, ppv, pbc = pools
                kt_max = 4 * (w + 1) if causal else NT
                wsl = slice(512 * w, 512 * (w + 1))
                for p in range(4):
                    pvs = [ppv.tile([65, 512], fp32, tag="pv", name="pv") for _ in range(2)]
                    exq = []

                    def pv_pair(kt):
                        ex2 = exq[kt]
                        for h in range(2):
                            nc.tensor.matmul(
                                pvs[h][:, :],
                                v_all[:, 130 * kt + 65 * h : 130 * kt + 65 * (h + 1)],
                                ex2[:, 512 * h : 512 * (h + 1)],
                                start=(kt == 0), stop=(kt == kt_max - 1))

                    for kt in range(kt_max):
                        dd = 128 * kt - 512 * w
                        sc2 = psc.tile([128, 1024], fp32, tag="sc2", name="sc2")
                        for h in range(2):
                            nc.tensor.matmul(
                                sc2[:, 512 * h : 512 * (h + 1)],
                                KT[64 * h : 64 * (h + 1), kt * 128 : (kt + 1) * 128],
                                QT[p][64 * h : 64 * (h + 1), wsl],
                                start=True, stop=True)
                        ex2 = aex.tile([128, 1024], bf16, tag="ex", name="ex")
                        nc.scalar.activation(out=ex2[:, :], in_=sc2[:, :], func=AF.Exp)
                        if causal and 0 <= dd <= 384:
                            off = 384 - dd
                            ex2v = ex2[:, :].rearrange("p (a b) -> p a b", a=2)
                            mrep = maskb[:, off : off + 512].rearrange(
                                "p (a f) -> p a f", a=1).to_broadcast((128, 2, 512))
                            nc.vector.tensor_mul(ex2v, ex2v, mrep)
                        exq.append(ex2)
                        if kt >= 1:
                            pv_pair(kt - 1)
                    pv_pair(kt_max - 1)

                    for h in range(2):
                        inv = asm.tile([1, 512], bf16, tag="inv", name="inv")
                        nc.vector.reciprocal(out=inv[:, :], in_=pvs[h][64:65, :])
                        bcp = pbc.tile([64, 512], fp32, tag="bcp", name="bcp")
                        nc.tensor.matmul(bcp[:, :], ones1b[0:1, 0:64], inv[0:1, :],
                                         start=True, stop=True)
                        bc = asm.tile([64, 512], bf16, tag="bc", name="bc")
                        nc.scalar.copy(out=bc[:, :], in_=bcp[:, :])
                        nc.vector.tensor_mul(AT[p][64 * h : 64 * (h + 1), wsl],
                                             pvs[h][0:64, :], bc[:, :])

            def oproj_w(w, po, xt=None, dc_rng=None, obw=None):
                wsl = slice(512 * w, 512 * (w + 1))
                if obw is None:
                    obw = obw_p.tile([128, NDC * 512], bf16, tag="obw", name="obw")
                if xt is None:
                    xt = xtw[w]
                if dc_rng is None:
                    dc_rng = range(NDC)
                for dc in dc_rng:
                    pso = po.tile([128, 512], fp32, tag="pso", name="pso")
                    for c in range(4):
                        nc.tensor.matmul(
                            pso[:, :],
                            wo_r[:, c * D + dc * 128 : c * D + (dc + 1) * 128],
                            AT[c][:, wsl], start=(c == 0), stop=(c == 3))
                    xsl = slice(512 * dc, 512 * (dc + 1))
                    t1 = oep.tile([128, 512], bf16, tag="t1", name="t1")
                    if has_pool_mul:
                        nc.gpsimd.tensor_mul(t1[:, :], xt[:, xsl], sbc_b[:, wsl])
                    else:
                        nc.vector.tensor_mul(t1[:, :], xt[:, xsl], sbc_b[:, wsl])
                    t2 = oep.tile([128, 512], bf16, tag="t2", name="t2")
                    nc.vector.tensor_scalar_mul(t2[:, :], t1[:, :], rmswT[:, dc : dc + 1])
                    nc.vector.tensor_add(obw[:, xsl], t2[:, :], pso[:, :])
                if dc_rng.stop == NDC:
                    nc.gpsimd.dma_start(
                        out=out[:, wsl].rearrange("(c p) t -> p c t", p=128),
                        in_=obw[:, :].rearrange("p (c t) -> p c t", c=NDC))
                return obw

            # ---- schedule ----
            load_xtw(0)
            if causal:
                for w in range(W512):
                    with (
                        tc.tile_pool(name="pq", bufs=4, space="PSUM") as pq,
                        tc.tile_pool(name="pkb", bufs=1, space="PSUM") as pkb,
                        tc.tile_pool(name="pv", bufs=1, space="PSUM") as pv_,
                    ):
                        qs, ks, vs = proj_w(w, (pq, pkb, pv_))
                        if w + 1 < W512:
                            load_xtw(w + 1)
                        stats_w(w, pkb)
                        rope_w(w, qs, ks, vs)
                    if w > 0:
                        with tc.tile_pool(name="po", bufs=4, space="PSUM") as po:
                            oproj_w(w - 1, po)
                    with (
                        tc.tile_pool(name="psc", bufs=2, space="PSUM") as psc,
                        tc.tile_pool(name="ppv", bufs=3, space="PSUM") as ppv,
                        tc.tile_pool(name="pbc", bufs=1, space="PSUM") as pbc,
                    ):
                        attn_w(w, (psc, ppv, pbc))
                with tc.tile_pool(name="po", bufs=4, space="PSUM") as po:
                    oproj_w(W512 - 1, po)
            else:
                for w in range(W512):
                    with (
                        tc.tile_pool(name="pq", bufs=4, space="PSUM") as pq,
                        tc.tile_pool(name="pkb", bufs=1, space="PSUM") as pkb,
                        tc.tile_pool(name="pv", bufs=1, space="PSUM") as pv_,
                    ):
                        qs, ks, vs = proj_w(w, (pq, pkb, pv_))
                        if w + 1 < W512:
                            load_xtw(w + 1)
                        stats_w(w, pkb)
                        rope_w(w, qs, ks, vs)
                # all windows' K/V needed before any attention window
                for w in range(W512):
                    with (
                        tc.tile_pool(name="psc", bufs=2, space="PSUM") as psc,
                        tc.tile_pool(name="ppv", bufs=3, space="PSUM") as ppv,
                        tc.tile_pool(name="pbc", bufs=1, space="PSUM") as pbc,
                    ):
                        attn_w(w, (psc, ppv, pbc))
                    # reload x^T for the residual (proj-time tiles are evicted)
                    ox = obw_p.tile([128, NDC * 512], bf16, tag="oxw", name="oxw")
                    wsl = slice(512 * w, 512 * (w + 1))
                    nc.gpsimd.dma_start(
                        out=ox[:, :].rearrange("p (c t) -> p c t", c=NDC),
                        in_=xT[:, wsl].rearrange("(c p) t -> p c t", p=128))
                    with tc.tile_pool(name="po", bufs=4, space="PSUM") as po:
                        oproj_w(w, po, xt=ox)
    return nc


def _host_prep(x, rms_w, Wq, Wk, Wv, Wo):
    import ml_dtypes
    f32 = np.float32
    bf16 = ml_dtypes.bfloat16
    x = np.asarray(x, f32)
    rms_w = np.asarray(rms_w, f32)
    wq_full = (np.asarray(Wq, f32) * rms_w[:, None] / math.sqrt(HD)).astype(f32)
    wk_full = (np.asarray(Wk, f32) * rms_w[:, None]).astype(f32)
    wv_full = (np.asarray(Wv, f32) * rms_w[:, None]).astype(f32)
    Wo = np.asarray(Wo, f32)

    inv_f = (1.0 / (10000.0 ** (np.arange(0, HD, 2, dtype=f32) / HD))).astype(f32)
    freqs = np.arange(S, dtype=f32)[:, None] * inv_f[None, :]   # [S, 32]
    cos = np.cos(freqs).astype(f32).T                           # [32, S]
    sin = np.sin(freqs).astype(f32).T
    cosT = np.tile(np.concatenate([cos, cos], 0), (2, 1))       # [128, S]
    sinT = np.tile(np.concatenate([-sin, sin], 0), (2, 1))

    kk = np.arange(128)[:, None]
    jj = np.arange(896)[None, :]
    maskb = (jj >= kk + 384).astype(f32)

    per_core = []
    for c in range(NC):
        b, g = c // 4, c % 4
        heads = [8 * g + h for h in PERM]
        wq_g = np.ascontiguousarray(
            np.concatenate([wq_full[:, 64 * h : 64 * (h + 1)] for h in heads], axis=1))
        wo_g = np.ascontiguousarray(
            np.concatenate([Wo[64 * h : 64 * (h + 1), :] for h in heads], axis=0))
        wk_g = np.ascontiguousarray(wk_full[:, 128 * g : 128 * (g + 1)])
        wv_g = np.ascontiguousarray(wv_full[:, 128 * g : 128 * (g + 1)])
        # chunk-major resident layouts: [128, chunk-index * cols]
        wq_r = np.ascontiguousarray(
            wq_g.reshape(NDC, 128, CQ).transpose(1, 0, 2).reshape(128, NDC * CQ))
        wk_r = np.ascontiguousarray(
            wk_g.reshape(NDC, 128, 128).transpose(1, 0, 2).reshape(128, NDC * 128))
        wv_r = np.ascontiguousarray(
            wv_g.reshape(NDC, 128, 128).transpose(1, 0, 2).reshape(128, NDC * 128))
        wo_r = np.ascontiguousarray(
            wo_g.reshape(4, 128, D).transpose(1, 0, 2).reshape(128, 4 * D))
        rmsw_g = rms_w if g == 0 else np.zeros((D,), f32)
        rmswT = np.ascontiguousarray(
            rmsw_g.reshape(NDC, 128).T.astype(f32))        # [128, NDC]
        xb = x[b].astype(bf16)
        per_core.append({
            "xT": np.ascontiguousarray(xb.T),
            "xb": np.ascontiguousarray(xb),
            "wq": wq_r.astype(bf16), "wk": wk_r.astype(bf16),
            "wv": wv_r.astype(bf16), "wo": wo_r.astype(bf16),
            "cosT": np.ascontiguousarray(cosT.astype(bf16)),
            "sinT": np.ascontiguousarray(sinT.astype(bf16)),
            "maskb": maskb.astype(bf16), "rmsw": rmswT,
        })
    return per_core


def kernel(x, rms_w, Wq, Wk, Wv, Wo, apply_causal_mask, _trace=False):
    from concourse import bass_utils
    _install_patch()
    causal = bool(int(np.asarray(apply_causal_mask)))
    if causal not in _cache:
        _cache[causal] = _build(causal)
    nc = _cache[causal]
    in_maps = _host_prep(x, rms_w, Wq, Wk, Wv, Wo)
    r = bass_utils.run_bass_kernel_spmd(nc, in_maps, core_ids=list(range(NC)),
                                        trace=_trace)
    outs = [np.asarray(r.results[c]["out"], dtype=np.float32) for c in range(NC)]
    full = np.stack([(outs[4 * b] + outs[4 * b + 1] + outs[4 * b + 2] + outs[4 * b + 3]).T
                     for b in range(B)]).astype(np.float32)
    if _trace:
        kernel.last_exec_time_ns = r.exec_time_ns
        kernel.last_result = r
    return full
